# revision 1
# baseline (speedup 1.0000x reference)
"""MemNN (end-to-end memory network) Trainium2 kernel.

All the heavy FLOPs of this network are six (B*L, V) @ (V, D) embedding
matmuls that share `facts` as LHS (A_h = facts @ Wa[h], C_h = facts @ Wc[h],
h = 0..2), plus one question embedding.  The six fuse into a single
(3200, 10000) @ (10000, 1536) matmul that does NOT depend on the hop
recurrence, so the whole 98.3 GFLOP is one bulk matmul.

Sharding: vocab (contraction) dim split 8 ways -> each core reads only its
1/8 slice of facts/Wa/Wc/Wq (~26 MB/core instead of ~88 MB/core with
replicated tables), computes a partial product at full PE rate (fp32r), and
writes it to DRAM.  The host unshards by summing the 8 partials (the unshard
step for partial-sum sharding) and runs the tiny sequential hop recurrence
(~0.03% of total FLOPs) in fp32.

Inputs are fp32; matmuls run in float32r (full fp32 values, full PE rate for
moving dim >= 256).
"""

import os

os.environ.setdefault("MYCRO_LOCAL_CACHE", "1")

import numpy as np

import concourse.bass as bass
import concourse.mybir as mybir
import concourse.tile as tile
from concourse.bass_utils import run_bass_kernel_spmd

HOPS, B, L, V, D = 3, 64, 50, 10000, 256
NCORES = 8
BL = B * L                # 3200 moving rows
NF = 2 * HOPS * D         # 1536 fused output cols: [Wa0|Wa1|Wa2|Wc0|Wc1|Wc2]
VSH = V // NCORES         # 1250 vocab rows per core
KT = 10                   # contraction tiles of 128 per core
VPAD = KT * 128           # 1280 (zero-padded)
MCH = 400                 # moving-col chunk; >=256 keeps fp32r at 1 cyc/row
NM = BL // MCH            # 8
NN = NF // 128            # 12 stationary W tiles
F32R = mybir.dt.float32r
F32 = mybir.dt.float32

_nc_cache = None
_last_result = None       # BassKernelResults of the most recent run (for profiling)


def _legalize_sync(nc):
    """Split multi-wait sync_info into standalone single-wait EventSemaphores.

    The walrus build in this environment enforces the raw-bass contract of at
    most ONE SyncWait per instruction ("Too many sync wait commands" in
    setupSyncWait otherwise), while Tile attaches every needed wait to the
    consuming instruction.  Hoisting all-but-one wait onto preceding
    InstEventSemaphore instructions on the same engine queue is semantically
    identical: engine queues are in-order, so a preceding wait blocks the
    queue exactly like an attached wait.  Updates are left untouched (they
    fire at completion and cannot be hoisted).
    """
    for func in nc.m.functions:
        for block in func.blocks:
            insts = list(block.instructions)
            out = []
            n = 0
            for inst in insts:
                si = inst.sync_info
                if si is not None and len(si.on_wait) > 1:
                    waits = list(si.on_wait)
                    for w in waits[:-1]:
                        ev = mybir.InstEventSemaphore(
                            name=f"{inst.name}-hoistw{n}", ins=[], outs=[]
                        )
                        n += 1
                        ev.engine = inst.engine
                        ev.sync_info = mybir.SyncInfo(on_wait=[w], on_update=[])
                        nc.register_instruction(ev)
                        out.append(ev)
                    inst.sync_info = mybir.SyncInfo(
                        on_wait=[waits[-1]], on_update=list(si.on_update)
                    )
                out.append(inst)
            if len(out) != len(insts):
                block.instructions = out
    return nc


# Moving-dim chunking of the 3200 BL columns.  All chunks >= 256 (fp32r runs
# 1 cyc/row only for moving dim >= 256).  The first chunk is narrow so the
# first matmul group's dependencies (one 128-col slice of wac + one facts
# chunk, ~2.7 MB) land quickly and the PE starts early.
_WIDTHS = [400] * 8
_STARTS = [sum(_WIDTHS[:i]) for i in range(len(_WIDTHS))]
assert sum(_WIDTHS) == BL


def _build(reps=1):
    """Build the SPMD device program.

    reps>1 repeats the main loop body (same data, same output addresses) —
    used only by the benchmark harness to measure device time differentially
    (per-call dispatch noise over the axon tunnel is ~ms, device time is
    ~200 us, so wall-clocking one launch cannot resolve it).
    """
    nc = bass.Bass(trn_type="TRN2")
    facts_t = nc.dram_tensor("facts_t", [VPAD, BL], F32R, kind="ExternalInput")
    wac = nc.dram_tensor("wac", [VPAD, NF], F32R, kind="ExternalInput")
    q_t = nc.dram_tensor("q_t", [VPAD, B], F32R, kind="ExternalInput")
    wq = nc.dram_tensor("wq", [VPAD, D], F32R, kind="ExternalInput")
    pac_t = nc.dram_tensor("pac_t", [NF, BL], F32, kind="ExternalOutput")
    pu = nc.dram_tensor("pu", [B, D], F32, kind="ExternalOutput")

    fr = facts_t.rearrange("(k p) n -> p k n", p=128)
    wr = wac.rearrange("(k p) n -> p k n", p=128)
    qr = q_t.rearrange("(k p) n -> p k n", p=128)
    wqr = wq.rearrange("(k p) n -> p k n", p=128)
    wmax = max(_WIDTHS)

    with (
        tile.TileContext(nc) as tc,
        tc.tile_pool(name="wpool", bufs=1) as wpool,
        tc.tile_pool(name="xpool", bufs=3) as xpool,
        tc.tile_pool(name="opool", bufs=4) as opool,
        tc.tile_pool(name="pspool", bufs=6, space="PSUM") as pspool,
    ):
        # Prologue DMA order: first wac n-slice 0 + first facts chunk (the
        # first matmul group's deps), then the rest of wac, then the small
        # question tensors.
        wt = wpool.tile([128, KT, NF], F32R)
        nc.sync.dma_start(wt[:, :, 0:128], wr[:, :, 0:128])
        xts = {}
        xts[0] = xpool.tile(
            [128, KT, _WIDTHS[0]], F32R, tag="xt", name="xt",
            padded_shape=[128, KT, wmax],
        )
        nc.sync.dma_start(xts[0][:], fr[:, :, 0 : _WIDTHS[0]])
        for off in range(128, NF, 512):
            end = min(off + 512, NF)
            nc.sync.dma_start(wt[:, :, off:end], wr[:, :, off:end])
        qtile = wpool.tile([128, KT, B], F32R)
        nc.sync.dma_start(qtile[:], qr)
        wqt = wpool.tile([128, KT, D], F32R)
        nc.sync.dma_start(wqt[:], wqr)

        def get_xt(mi):
            if mi not in xts:
                xts[mi] = xpool.tile(
                    [128, KT, _WIDTHS[mi]], F32R, tag="xt", name="xt",
                    padded_shape=[128, KT, wmax],
                )
                nc.sync.dma_start(
                    xts[mi][:], fr[:, :, _STARTS[mi] : _STARTS[mi] + _WIDTHS[mi]]
                )
            return xts[mi]

        # Main fused matmul: out(n, m) += sum_k wac[k, n].T @ facts_t[k, m]
        for _ in range(reps):
            for mi in range(len(_WIDTHS)):
                xt = get_xt(mi)
                for n in range(NN):
                    ps = pspool.tile(
                        [128, _WIDTHS[mi]], F32, tag="ps", name="ps",
                        padded_shape=[128, wmax],
                    )
                    for k in range(KT):
                        nc.tensor.matmul(
                            ps[:],
                            wt[:, k, n * 128 : (n + 1) * 128],
                            xt[:, k, :],
                            start=(k == 0),
                            stop=(k == KT - 1),
                        )
                    ot = opool.tile(
                        [128, _WIDTHS[mi]], F32, tag="ot", name="ot",
                        padded_shape=[128, wmax],
                    )
                    nc.vector.tensor_copy(ot[:], ps[:])
                    nc.sync.dma_start(
                        pac_t[
                            n * 128 : (n + 1) * 128,
                            _STARTS[mi] : _STARTS[mi] + _WIDTHS[mi],
                        ],
                        ot[:],
                    )
            xts.clear()

        # Question embedding at the tail: its PE work (10 small matmuls)
        # overlaps the main loop's epilogue.
        psq = pspool.tile([B, D], F32, tag="psq", bufs=1)
        for k in range(KT):
            nc.tensor.matmul(
                psq[:], qtile[:, k, :], wqt[:, k, :], start=(k == 0), stop=(k == KT - 1)
            )
        uo = opool.tile([B, D], F32, tag="uo")
        nc.any.tensor_copy(out=uo[:], in_=psq[:])
        nc.sync.dma_start(pu[:, :], uo[:])
    return _legalize_sync(nc)


def _shard_inputs(facts, question, Wq, Wa, Wc):
    fx = np.ascontiguousarray(facts, dtype=np.float32).reshape(BL, V)
    qx = np.asarray(question, dtype=np.float32).sum(axis=1)  # (B, V) bag-of-words
    Wq = np.asarray(Wq, dtype=np.float32)
    Wa = np.asarray(Wa, dtype=np.float32)
    Wc = np.asarray(Wc, dtype=np.float32)
    wac_full = np.concatenate([Wa[0], Wa[1], Wa[2], Wc[0], Wc[1], Wc[2]], axis=1)

    in_maps = []
    for c in range(NCORES):
        sl = slice(c * VSH, (c + 1) * VSH)
        ft = np.zeros((VPAD, BL), np.float32)
        ft[:VSH] = fx[:, sl].T
        qt = np.zeros((VPAD, B), np.float32)
        qt[:VSH] = qx[:, sl].T
        ws = np.zeros((VPAD, NF), np.float32)
        ws[:VSH] = wac_full[sl]
        wqs = np.zeros((VPAD, D), np.float32)
        wqs[:VSH] = Wq[sl]
        in_maps.append({"facts_t": ft, "q_t": qt, "wac": ws, "wq": wqs})
    return in_maps


def _wait_for_devices(min_wait_attempts=10):
    """The axon terminal occasionally reports a transient bad topology
    ("terminal has 1 core"); poll until all 8 NeuronCores are visible."""
    import time as _time

    import jax

    for attempt in range(min_wait_attempts):
        try:
            if len(jax.devices()) >= NCORES:
                return
        except Exception:  # noqa: BLE001 - backend init failure is retryable
            try:
                jax.clear_backends()
            except Exception:  # noqa: BLE001
                pass
        _time.sleep(15.0)
    # fall through: let the run itself raise a descriptive error


def _run_with_retries(nc, in_maps, attempts=4):
    """run_bass_kernel_spmd with retries: the axon terminal occasionally
    reports transient failures (device wedged / NRT_EXEC_UNIT_UNRECOVERABLE /
    temporary topology glitches) that succeed on re-dispatch."""
    import time as _time

    last_exc = None
    for attempt in range(attempts):
        try:
            return run_bass_kernel_spmd(nc, in_maps, list(range(NCORES)))
        except Exception as e:  # noqa: BLE001 - retry any runtime failure
            last_exc = e
            if attempt < attempts - 1:
                _time.sleep(10.0 * (attempt + 1))
                _wait_for_devices(min_wait_attempts=4)
    raise last_exc


def kernel(facts, question, Wq, Wa, Wc, Ww, bw):
    global _nc_cache, _last_result
    _wait_for_devices(min_wait_attempts=8)
    in_maps = _shard_inputs(facts, question, Wq, Wa, Wc)
    if _nc_cache is None:
        _nc_cache = _build()
    _last_result = _run_with_retries(_nc_cache, in_maps)
    res = _last_result.results

    # Unshard: sum the 8 partial products of the vocab-sharded matmul.
    ac_t = res[0]["pac_t"].copy()
    u = res[0]["pu"].copy()
    for r in res[1:]:
        ac_t += r["pac_t"]
        u += r["pu"]

    # Sequential hop recurrence (tiny: ~30 MFLOP vs 98.3 GFLOP on device).
    Ww = np.asarray(Ww, dtype=np.float32)
    bw = np.asarray(bw, dtype=np.float32)
    for h in range(HOPS):
        A = ac_t[h * D : (h + 1) * D].reshape(D, B, L)
        C = ac_t[(HOPS + h) * D : (HOPS + h + 1) * D].reshape(D, B, L)
        match = np.einsum("dbl,bd->bl", A, u)
        mm = match - match.max(axis=-1, keepdims=True)
        e = np.exp(mm)
        p = e / e.sum(axis=-1, keepdims=True)
        att = np.einsum("bl,dbl->bd", p, C)
        z = (u + att) @ Ww[h] + bw[h]
        if h == HOPS - 1:
            zz = z - z.max(axis=-1, keepdims=True)
            ez = np.exp(zz)
            u = ez / ez.sum(axis=-1, keepdims=True)
        else:
            u = np.maximum(z, 0.0)
    return np.ascontiguousarray(u, dtype=np.float32)



# revision 2
# speedup vs baseline: 1.7714x; 1.7714x over previous
"""MemNN (end-to-end memory network) Trainium2 kernel.

All the heavy FLOPs are six (B*L, V) @ (V, D) embedding matmuls sharing
`facts` as LHS (A_h = facts @ Wa[h], C_h = facts @ Wc[h]), fused into one
(3200, 10000) @ (10000, 1536) matmul independent of the hop recurrence.

Sharding: vocab (contraction) split 8 ways; each core computes a partial
product, host sums the 8 partials and runs the tiny hop recurrence.

Precision schedule (tolerance is 2e-2 relative):
 - A-half (768 cols): bf16 x bf16 matmul.  A feeds the attention logits
   match_h = A_h . u_h, whose noise the softmax amplifies -- fp8 here fails.
 - C-half (768 cols): fp8 e4m3 with MatmulPerfMode.DoubleRow (2 contraction
   tiles per instruction, 2x PE rate).  C only enters through the smooth
   p-weighted average, so fp8 noise is tolerable.  facts are mean-shifted
   (f - 0.5) before quantizing, halving quantization noise; the exact
   rank-1 correction 0.5*colsum(Wc) is added to `att` on the host (p sums
   to 1, so it is a constant vector add).  Wc is pre-scaled by 2^11 so its
   ~0.02-magnitude entries land in fp8's normal range; the host divides
   the C partials by 2^11.
 - Question embedding (0.3% of FLOPs): exact fp32r, tail of the kernel.

End-to-end relative error ~8e-3 (numpy-simulated on the exact inputs),
vs 2.1e-1 budget headroom at fp32r's 1.9e-4.  PE cost per core drops from
120 slot-units (12 n-tiles x 10 k) to 90 (A: 6x10 bf16, C: 6x5 DoubleRow).
"""

import os

os.environ.setdefault("MYCRO_LOCAL_CACHE", "1")

import ml_dtypes
import numpy as np

import concourse.bass as bass
import concourse.mybir as mybir
import concourse.tile as tile
from concourse.bass_utils import run_bass_kernel_spmd

HOPS, B, L, V, D = 3, 64, 50, 10000, 256
NCORES = 8
BL = B * L                # 3200 moving rows
NA = HOPS * D             # 768 A cols: [Wa0|Wa1|Wa2]
NF = 2 * HOPS * D         # 1536 total output rows of pac_t
VSH = V // NCORES         # 1250 vocab rows per core
KT = 10                   # contraction tiles of 128 per core
VPAD = KT * 128           # 1280 (zero-padded)
MCH = 400                 # moving-col chunk; >=256 keeps full PE rate
NN = NA // 128            # 6 stationary tiles per half
WSC = 2048.0              # 2^11 Wc pre-scale for fp8
F32R = mybir.dt.float32r
F32 = mybir.dt.float32
BF16 = mybir.dt.bfloat16
FP8 = mybir.dt.float8e4
NP_BF16 = ml_dtypes.bfloat16
NP_FP8 = ml_dtypes.float8_e4m3fn

_nc_cache = None
_last_result = None       # BassKernelResults of the most recent run (for profiling)


def _legalize_sync(nc):
    """Split multi-wait sync_info into standalone single-wait EventSemaphores.

    The walrus build in this environment enforces the raw-bass contract of at
    most ONE SyncWait per instruction ("Too many sync wait commands" in
    setupSyncWait otherwise), while Tile attaches every needed wait to the
    consuming instruction.  Hoisting all-but-one wait onto preceding
    InstEventSemaphore instructions on the same engine queue is semantically
    identical: engine queues are in-order, so a preceding wait blocks the
    queue exactly like an attached wait.  Updates are left untouched (they
    fire at completion and cannot be hoisted).
    """
    for func in nc.m.functions:
        for block in func.blocks:
            insts = list(block.instructions)
            out = []
            n = 0
            for inst in insts:
                si = inst.sync_info
                if si is not None and len(si.on_wait) > 1:
                    waits = list(si.on_wait)
                    for w in waits[:-1]:
                        ev = mybir.InstEventSemaphore(
                            name=f"{inst.name}-hoistw{n}", ins=[], outs=[]
                        )
                        n += 1
                        ev.engine = inst.engine
                        ev.sync_info = mybir.SyncInfo(on_wait=[w], on_update=[])
                        nc.register_instruction(ev)
                        out.append(ev)
                    inst.sync_info = mybir.SyncInfo(
                        on_wait=[waits[-1]], on_update=list(si.on_update)
                    )
                out.append(inst)
            if len(out) != len(insts):
                block.instructions = out
    return nc


_WIDTHS = [MCH] * (BL // MCH)
_STARTS = [sum(_WIDTHS[:i]) for i in range(len(_WIDTHS))]
assert sum(_WIDTHS) == BL


def _build(reps=1):
    """Build the SPMD device program.

    reps>1 repeats the main loop body (same data, same output addresses) --
    used only by the benchmark harness to measure device time differentially
    (per-call dispatch noise over the axon tunnel is ~ms, device time is
    ~200 us, so wall-clocking one launch cannot resolve it).
    """
    nc = bass.Bass(trn_type="TRN2")
    facts_b = nc.dram_tensor("facts_b", [VPAD, BL], BF16, kind="ExternalInput")
    facts_8 = nc.dram_tensor("facts_8", [VPAD, BL], FP8, kind="ExternalInput")
    wa_b = nc.dram_tensor("wa_b", [VPAD, NA], BF16, kind="ExternalInput")
    wc_8 = nc.dram_tensor("wc_8", [VPAD, NA], FP8, kind="ExternalInput")
    q_t = nc.dram_tensor("q_t", [VPAD, B], F32R, kind="ExternalInput")
    wq = nc.dram_tensor("wq", [VPAD, D], F32R, kind="ExternalInput")
    pac_t = nc.dram_tensor("pac_t", [NF, BL], F32, kind="ExternalOutput")
    pu = nc.dram_tensor("pu", [B, D], F32, kind="ExternalOutput")

    fbr = facts_b.rearrange("(k p) n -> p k n", p=128)
    f8r = facts_8.rearrange("(k p) n -> p k n", p=128)
    war = wa_b.rearrange("(k p) n -> p k n", p=128)
    wcr = wc_8.rearrange("(k p) n -> p k n", p=128)
    qr = q_t.rearrange("(k p) n -> p k n", p=128)
    wqr = wq.rearrange("(k p) n -> p k n", p=128)
    wmax = max(_WIDTHS)

    with (
        tile.TileContext(nc) as tc,
        tc.tile_pool(name="wpool", bufs=1) as wpool,
        tc.tile_pool(name="xbpool", bufs=3) as xbpool,
        tc.tile_pool(name="x8pool", bufs=3) as x8pool,
        tc.tile_pool(name="opool", bufs=4) as opool,
        tc.tile_pool(name="pspool", bufs=6, space="PSUM") as pspool,
    ):
        # Prologue DMA order: first wa_b n-slice 0 + first bf16 facts chunk
        # (the first matmul group's deps), then the remaining weights, then
        # the small question tensors.
        wat = wpool.tile([128, KT, NA], BF16)
        nc.sync.dma_start(wat[:, :, 0:128], war[:, :, 0:128])
        xbs = {}
        x8s = {}
        xbs[0] = xbpool.tile(
            [128, KT, _WIDTHS[0]], BF16, tag="xb", name="xb",
            padded_shape=[128, KT, wmax],
        )
        nc.sync.dma_start(xbs[0][:], fbr[:, :, 0 : _WIDTHS[0]])
        for off in range(128, NA, 320):
            end = min(off + 320, NA)
            nc.sync.dma_start(wat[:, :, off:end], war[:, :, off:end])
        wct = wpool.tile([128, KT, NA], FP8)
        for off in range(0, NA, 384):
            end = min(off + 384, NA)
            nc.sync.dma_start(wct[:, :, off:end], wcr[:, :, off:end])
        x8s[0] = x8pool.tile(
            [128, KT, _WIDTHS[0]], FP8, tag="x8", name="x8",
            padded_shape=[128, KT, wmax],
        )
        nc.sync.dma_start(x8s[0][:], f8r[:, :, 0 : _WIDTHS[0]])
        qtile = wpool.tile([128, KT, B], F32R)
        nc.sync.dma_start(qtile[:], qr)
        wqt = wpool.tile([128, KT, D], F32R)
        nc.sync.dma_start(wqt[:], wqr)

        def get_xt(mi, xs, pool, dt, rr, tg):
            if mi not in xs:
                xs[mi] = pool.tile(
                    [128, KT, _WIDTHS[mi]], dt, tag=tg, name=tg,
                    padded_shape=[128, KT, wmax],
                )
                nc.sync.dma_start(
                    xs[mi][:], rr[:, :, _STARTS[mi] : _STARTS[mi] + _WIDTHS[mi]]
                )
            return xs[mi]

        def drain(ps, n, mi):
            ot = opool.tile(
                [128, _WIDTHS[mi]], F32, tag="ot", name="ot",
                padded_shape=[128, wmax],
            )
            nc.vector.tensor_copy(ot[:], ps[:])
            nc.sync.dma_start(
                pac_t[
                    n * 128 : (n + 1) * 128,
                    _STARTS[mi] : _STARTS[mi] + _WIDTHS[mi],
                ],
                ot[:],
            )

        # Main fused matmul.  A-half: out(n,m) += sum_k wa[k,n].T @ fb[k,m]
        # in bf16.  C-half: fp8 DoubleRow, two k-tiles per instruction.
        for _ in range(reps):
            for mi in range(len(_WIDTHS)):
                xb = get_xt(mi, xbs, xbpool, BF16, fbr, "xb")
                x8 = get_xt(mi, x8s, x8pool, FP8, f8r, "x8")
                for n in range(NN):
                    ps = pspool.tile(
                        [128, _WIDTHS[mi]], F32, tag="ps", name="ps",
                        padded_shape=[128, wmax],
                    )
                    for k in range(KT):
                        nc.tensor.matmul(
                            ps[:],
                            wat[:, k, n * 128 : (n + 1) * 128],
                            xb[:, k, :],
                            start=(k == 0),
                            stop=(k == KT - 1),
                        )
                    drain(ps, n, mi)
                for n in range(NN):
                    ps = pspool.tile(
                        [128, _WIDTHS[mi]], F32, tag="ps", name="ps",
                        padded_shape=[128, wmax],
                    )
                    for t in range(KT // 2):
                        nc.tensor.matmul(
                            ps[:],
                            wct[:, 2 * t : 2 * t + 2, n * 128 : (n + 1) * 128],
                            x8[:, 2 * t : 2 * t + 2, :],
                            start=(t == 0),
                            stop=(t == KT // 2 - 1),
                            perf_mode=mybir.MatmulPerfMode.DoubleRow,
                        )
                    drain(ps, NN + n, mi)
            xbs.clear()
            x8s.clear()

        # Question embedding at the tail: its PE work (10 small matmuls)
        # overlaps the main loop's epilogue.
        psq = pspool.tile([B, D], F32, tag="psq", bufs=1)
        for k in range(KT):
            nc.tensor.matmul(
                psq[:], qtile[:, k, :], wqt[:, k, :], start=(k == 0), stop=(k == KT - 1)
            )
        uo = opool.tile([B, D], F32, tag="uo")
        nc.any.tensor_copy(out=uo[:], in_=psq[:])
        nc.sync.dma_start(pu[:, :], uo[:])
    return _legalize_sync(nc)


def _shard_inputs(facts, question, Wq, Wa, Wc):
    fx = np.ascontiguousarray(facts, dtype=np.float32).reshape(BL, V)
    fxb = fx.astype(NP_BF16)
    fx8 = (fx - np.float32(0.5)).astype(NP_FP8)
    qx = np.asarray(question, dtype=np.float32).sum(axis=1)  # (B, V) bag-of-words
    Wq = np.asarray(Wq, dtype=np.float32)
    Wa = np.asarray(Wa, dtype=np.float32)
    Wc = np.asarray(Wc, dtype=np.float32)
    wa_full = np.concatenate([Wa[0], Wa[1], Wa[2]], axis=1).astype(NP_BF16)
    wc_full = (
        np.concatenate([Wc[0], Wc[1], Wc[2]], axis=1) * np.float32(WSC)
    ).astype(NP_FP8)

    in_maps = []
    for c in range(NCORES):
        sl = slice(c * VSH, (c + 1) * VSH)
        fb = np.zeros((VPAD, BL), NP_BF16)
        fb[:VSH] = fxb[:, sl].T
        f8 = np.zeros((VPAD, BL), NP_FP8)
        f8[:VSH] = fx8[:, sl].T
        qt = np.zeros((VPAD, B), np.float32)
        qt[:VSH] = qx[:, sl].T
        wab = np.zeros((VPAD, NA), NP_BF16)
        wab[:VSH] = wa_full[sl]
        wc8 = np.zeros((VPAD, NA), NP_FP8)
        wc8[:VSH] = wc_full[sl]
        wqs = np.zeros((VPAD, D), np.float32)
        wqs[:VSH] = Wq[sl]
        in_maps.append(
            {"facts_b": fb, "facts_8": f8, "q_t": qt, "wa_b": wab,
             "wc_8": wc8, "wq": wqs}
        )
    return in_maps


def _wait_for_devices(min_wait_attempts=10):
    """The axon terminal occasionally reports a transient bad topology
    ("terminal has 1 core"); poll until all 8 NeuronCores are visible."""
    import time as _time

    import jax

    for attempt in range(min_wait_attempts):
        try:
            if len(jax.devices()) >= NCORES:
                return
        except Exception:  # noqa: BLE001 - backend init failure is retryable
            try:
                jax.clear_backends()
            except Exception:  # noqa: BLE001
                pass
        _time.sleep(15.0)
    # fall through: let the run itself raise a descriptive error


def _run_with_retries(nc, in_maps, attempts=4):
    """run_bass_kernel_spmd with retries: the axon terminal occasionally
    reports transient failures (device wedged / NRT_EXEC_UNIT_UNRECOVERABLE /
    temporary topology glitches) that succeed on re-dispatch."""
    import time as _time

    last_exc = None
    for attempt in range(attempts):
        try:
            return run_bass_kernel_spmd(nc, in_maps, list(range(NCORES)))
        except Exception as e:  # noqa: BLE001 - retry any runtime failure
            last_exc = e
            if attempt < attempts - 1:
                _time.sleep(10.0 * (attempt + 1))
                _wait_for_devices(min_wait_attempts=4)
    raise last_exc


def kernel(facts, question, Wq, Wa, Wc, Ww, bw):
    global _nc_cache, _last_result
    _wait_for_devices(min_wait_attempts=8)
    in_maps = _shard_inputs(facts, question, Wq, Wa, Wc)
    if _nc_cache is None:
        _nc_cache = _build()
    _last_result = _run_with_retries(_nc_cache, in_maps)
    res = _last_result.results

    # Unshard: sum the 8 partial products of the vocab-sharded matmul.
    ac_t = res[0]["pac_t"].copy()
    u = res[0]["pu"].copy()
    for r in res[1:]:
        ac_t += r["pac_t"]
        u += r["pu"]

    Wc = np.asarray(Wc, dtype=np.float32)
    colsum_wc = Wc.sum(axis=1)  # (HOPS, D): exact rank-1 shift correction

    # Sequential hop recurrence (tiny: ~30 MFLOP vs 98.3 GFLOP on device).
    Ww = np.asarray(Ww, dtype=np.float32)
    bw = np.asarray(bw, dtype=np.float32)
    for h in range(HOPS):
        A = ac_t[h * D : (h + 1) * D].reshape(D, B, L)
        C = ac_t[(HOPS + h) * D : (HOPS + h + 1) * D].reshape(D, B, L)
        match = np.einsum("dbl,bd->bl", A, u)
        mm = match - match.max(axis=-1, keepdims=True)
        e = np.exp(mm)
        p = e / e.sum(axis=-1, keepdims=True)
        # C partials carry the 2^11 fp8 pre-scale; p sums to 1, so the
        # mean-shift correction is a constant vector add.
        att = np.einsum("bl,dbl->bd", p, C) * np.float32(1.0 / WSC)
        att += np.float32(0.5) * colsum_wc[h]
        z = (u + att) @ Ww[h] + bw[h]
        if h == HOPS - 1:
            zz = z - z.max(axis=-1, keepdims=True)
            ez = np.exp(zz)
            u = ez / ez.sum(axis=-1, keepdims=True)
        else:
            u = np.maximum(z, 0.0)
    return np.ascontiguousarray(u, dtype=np.float32)
